# revision 6
# baseline (speedup 1.0000x reference)
"""Trainium2 Bass kernel for a 2-layer LSTM (MnistModel frames).

Model: xb [2048, 8192] -> frames [T=64, B, 128] -> LSTM(128->512) ->
LSTM(512->512) -> last hidden -> Linear(512->10).

Sharding: data-parallel over batch (2048 -> 256 per core, 8 cores),
weights replicated.  Everything on-chip is transposed ([feature,
batch]).

Matmuls run in fp8e4m3 with DoubleRow perf mode (2 k-chunks of 128 per
instruction) with full residual compensation so the quantization error
stays ~2e-3:
  PSUM scale S=32.  Per hidden-path, three stationary variants:
    Qw32 = fp8(32 W), Qw2 = fp8(2 W), Rw32 = fp8(32 (W - Qw32/32))
  Moving h is kept as (Qh = fp8(h), Rh16 = fp8(16 (h - Qh))).
  act pair (k):     (Qh_k, Rh16_k) @ (Qw32_k, Qw2_k)
  W pair (k0,k1):   (Qh_k0, Qh_k1) @ (Rw32_k0, Rw32_k1)
  bias pair:        (ones_row, 0)  @ (fp8(32 b) row plane, 0)
  x (layer 0) runs as one exact f32r matmul with weights 32*W_ih0.
Gates accumulate per (gate, p) region of gate-major PSUM tiles
[128, 4, 256]; ACT applies sigmoid/tanh with scale=1/32 in one wide
instruction per gate tile; elementwise runs bf16 on DVE (c state fp32,
c-update on GPSIMD/Pool).
"""

import os
import sys

import numpy as np

for _p in ("/opt/trn_rl_repo", "/root/.axon_site/_ro/trn_rl_repo"):
    if os.path.isdir(_p) and _p not in sys.path:
        sys.path.insert(0, _p)

import concourse.bass as bass  # noqa: E402
import concourse.mybir as mybir  # noqa: E402
import concourse.tile as tile  # noqa: E402
from concourse import bacc  # noqa: E402
from concourse.bass_utils import run_bass_kernel_spmd  # noqa: E402
from concourse.masks import make_identity  # noqa: E402

F32 = mybir.dt.float32
F32R = mybir.dt.float32r
F8 = mybir.dt.float8e4
BF16 = mybir.dt.bfloat16
AF = mybir.ActivationFunctionType
DR = mybir.MatmulPerfMode.DoubleRow

B, L, IN, H, OUT = 2048, 8192, 128, 512, 10
T = L // IN  # 64 timesteps
NCORES = 8
BL = B // NCORES  # 256 batch rows per core
G4 = 4 * H  # 2048 gate rows
NKC = H // 128  # 4 hidden k-chunks
NB = BL  # matmul moving free dim (batch)
S = 32.0  # PSUM gate scale
GIDX = {"i": 0, "f": 1, "g": 2, "o": 3}  # gate order in W rows (pytorch)

_CACHE = {}


def _build(opts=None):
    _defaults = dict(
        lag=2,
        xt_sbuf=4,
        comp_w=True,   # emit Rw32 W-pairs
        comp_a=True,   # emit Rh16 act-residual planes (else zero plane)
    )
    _defaults.update(opts or {})
    opts = _defaults
    LAG = opts["lag"]
    SKIP = opts["xt_sbuf"]
    COMP_W = opts["comp_w"]
    COMP_A = opts["comp_a"]

    nc = bacc.Bacc()
    xb = nc.declare_dram_parameter("xb", [BL, L], F32, isOutput=False)
    W_ih0 = nc.declare_dram_parameter("W_ih0", [G4, IN], F32, isOutput=False)
    W_hh0 = nc.declare_dram_parameter("W_hh0", [G4, H], F32, isOutput=False)
    b0 = nc.declare_dram_parameter("b0", [G4], F32, isOutput=False)
    W_ih1 = nc.declare_dram_parameter("W_ih1", [G4, H], F32, isOutput=False)
    W_hh1 = nc.declare_dram_parameter("W_hh1", [G4, H], F32, isOutput=False)
    b1 = nc.declare_dram_parameter("b1", [G4], F32, isOutput=False)
    W_out = nc.declare_dram_parameter("W_out", [OUT, H], F32, isOutput=False)
    b_out = nc.declare_dram_parameter("b_out", [OUT], F32, isOutput=False)
    out = nc.declare_dram_parameter("out", [BL, OUT], F32, isOutput=True)

    with tile.TileContext(nc) as tc:
        with (
            tc.tile_pool(name="const", bufs=1) as const,
            tc.tile_pool(name="xt_dram", bufs=1, space="DRAM") as xdp,
            tc.tile_pool(name="wstg", bufs=2) as wstg,
            tc.tile_pool(name="psg", bufs=4, space="PSUM") as psg,
        ):
            ident = const.tile([128, 128], F32, tag="ident")
            make_identity(nc, ident)
            # fp8 -1/32 "identity" for residual computation on the PE
            nident = const.tile([128, 128], F8, tag="nident")
            nc.vector.tensor_scalar_mul(nident, ident, -1.0 / S)

            # ones/zero moving pair for bias injection
            om = const.tile([128, 2, 256], F8, tag="om")
            nc.vector.memset(om, 0.0)
            nc.vector.memset(om[0:1, 0, :], 1.0)

            # bias row planes: [128, 2, G4] fp8, partition0/plane0 = 32*b
            def bias_plane(bsrc, name):
                bp = const.tile([128, 2, G4], F8, tag=name, name=name)
                nc.vector.memset(bp, 0.0)
                stb = wstg.tile([1, G4], F32, tag="st", name=f"st_{name}")
                nc.sync.dma_start(
                    out=stb, in_=bsrc[:].rearrange("(o m) -> o m", o=1)
                )
                nc.vector.tensor_scalar_mul(bp[0:1, 0, :], stb, S)
                return bp

            bp0 = bias_plane(b0, "bp0")
            bp1 = bias_plane(b1, "bp1")

            bot = const.tile([OUT, 1], F32, tag="bot")
            nc.sync.dma_start(out=bot, in_=b_out[:].rearrange("(p o) -> p o", o=1))

            # ---- fp8 weight prep for one hidden path ([G4, H] weights) ----
            # produces wq [128, NKC, 2, G4] (planes Qw32, Qw2) and
            # wr [128, NKC, G4] (Rw32).
            def load_w_fp8(wd, name):
                wq = const.tile(
                    [128, NKC, 2, G4], F8, tag=f"wq_{name}", name=f"wq_{name}"
                )
                wr = const.tile(
                    [128, NKC, G4], F8, tag=f"wr_{name}", name=f"wr_{name}"
                )
                wv = wd[:, :].rearrange("(g p) k -> p g k", p=128)
                for qt in range(4):
                    # stage 4 m-chunks (quarter of gate rows)
                    st = wstg.tile(
                        [128, 4, H], F32, tag="st", name=f"st_{name}{qt}"
                    )
                    nc.scalar.dma_start(out=st, in_=wv[:, qt * 4 : qt * 4 + 4, :])
                    for kc in range(NKC):
                        # transpose 4 m-blocks of this k-chunk into one
                        # [128, 512] half of a psum tile (one bank)
                        pt = psg.tile([128, NKC, NB], F32, tag="g", name="wpt")
                        ptf = pt.rearrange("p a b -> p (a b)")[:, 0:512]
                        for mg in range(4):
                            nc.tensor.matmul(
                                ptf[:, mg * 128 : (mg + 1) * 128],
                                st[:, mg, kc * 128 : kc * 128 + 128],
                                ident,
                                is_transpose=True,
                                start=(mg == 0),
                                stop=False,
                                skip_group_check=True,
                            )
                        mlo = qt * 512
                        # Qw32 / Qw2 casts
                        nc.scalar.activation(
                            wq[:, kc, 0, mlo : mlo + 512], ptf, AF.Copy, scale=S
                        )
                        nc.vector.tensor_scalar_mul(
                            wq[:, kc, 1, mlo : mlo + 512], ptf, 2.0
                        )
                        # residual: psum += (-1/32) * Qw32  (on the PE)
                        nc.tensor.matmul(
                            ptf,
                            nident,
                            wq[:, kc, 0, mlo : mlo + 512],
                            start=False,
                            stop=True,
                            skip_group_check=True,
                        )
                        if COMP_W:
                            nc.vector.tensor_scalar_mul(
                                wr[:, kc, mlo : mlo + 512], ptf, S
                            )
                return wq, wr

            # ---- x transpose machinery (f32r, exact), baseline-style ----
            CHUNK_T = 8
            xts_d = [
                None
                if t < SKIP
                else xdp.tile([128, NB], F32R, tag=f"xt{t}", name=f"xtd{t}")
                for t in range(T)
            ]
            xts_sb = [
                const.tile([128, NB], F32R, tag=f"xts{t}", name=f"xts{t}")
                if t < SKIP
                else None
                for t in range(T)
            ]

            def x_phase(xstg, xsb):
                for tch in range(0, T, CHUNK_T):
                    tend = min(tch + CHUNK_T, T)
                    lo, hi = tch * IN, tend * IN
                    xs0 = xstg.tile([128, CHUNK_T * IN], F32, tag="xs0", name="xs0")
                    xs1 = xstg.tile([128, CHUNK_T * IN], F32, tag="xs1", name="xs1")
                    nc.sync.dma_start(out=xs0, in_=xb[0:128, lo:hi])
                    nc.sync.dma_start(out=xs1, in_=xb[128:256, lo:hi])
                    for t in range(tch, tend):
                        off = (t - tch) * IN
                        pt = psg.tile([128, 4, 256], F32, tag="g", name="xpt")
                        ptf = pt.rearrange("p a b -> p (a b)")
                        nc.tensor.matmul(
                            ptf[:, 0:128], xs0[:, off : off + IN], ident,
                            is_transpose=True, start=True, stop=False,
                            skip_group_check=True,
                        )
                        nc.tensor.matmul(
                            ptf[:, 128:256], xs1[:, off : off + IN], ident,
                            is_transpose=True, start=False, stop=True,
                            skip_group_check=True,
                        )
                        if t < SKIP:
                            nc.vector.tensor_copy(xts_sb[t], ptf[:, 0:NB])
                        else:
                            sb = xsb.tile([128, NB], F32R, tag="sb", name="xsb")
                            nc.vector.tensor_copy(sb, ptf[:, 0:NB])
                            nc.sync.dma_start(out=xts_d[t][:, :], in_=sb)

            # ---- startup: layer-0 weights + x transposes ----
            # WT_ih0 * 32 as f32r [128, G4]
            WT_ih0 = const.tile([128, G4], F32R, tag="wih0")
            stw0 = wstg.tile([128, 16, IN], F32, tag="st", name="st_wih0")
            nc.scalar.dma_start(
                out=stw0, in_=W_ih0[:, :].rearrange("(g p) k -> p g k", p=128)
            )
            for mg in range(4):
                pt = psg.tile([128, NKC, NB], F32, tag="g", name="wpt0")
                ptf = pt.rearrange("p a b -> p (a b)")
                for j in range(4):
                    nc.tensor.matmul(
                        ptf[:, j * 128 : (j + 1) * 128],
                        stw0[:, mg * 4 + j, :],
                        ident,
                        is_transpose=True,
                        start=(j == 0),
                        stop=(j == 3),
                        skip_group_check=True,
                    )
                nc.vector.tensor_scalar_mul(
                    WT_ih0[:, mg * 512 : (mg + 1) * 512], ptf[:, 0:512], S
                )

            wq0, wr0 = load_w_fp8(W_hh0, "hh0")

            with (
                tc.tile_pool(name="xstg", bufs=2) as xstg,
                tc.tile_pool(name="xsb", bufs=4) as xsb,
            ):
                x_phase(xstg, xsb)

            # ---- recurrence ----
            with (
                tc.tile_pool(name="xtp", bufs=3) as xtp,
                tc.tile_pool(name="state", bufs=2) as stp,
                tc.tile_pool(name="work", bufs=2) as wkp,
            ):
                # initial states
                def zero_h(name, bufs):
                    tl = stp.tile(
                        [128, 2, NKC, NB], F8, tag=name, name=name, bufs=bufs
                    )
                    nc.vector.memset(tl, 0.0)
                    return tl

                h0 = zero_h("h0", LAG + 2)
                h1 = zero_h("h1", 2)
                c0 = stp.tile([128, NKC * NB], F32, tag="c0", bufs=2)
                nc.vector.memset(c0, 0.0)
                c1 = stp.tile([128, NKC * NB], F32, tag="c1", bufs=2)
                nc.vector.memset(c1, 0.0)
                state = {"h0": h0, "c0": c0, "h1": h1, "c1": c1}

                hs0 = {}  # t -> h0 tile for layer 1

                def chain_l0(reg, mo, xt, hq):
                    """All matmuls for one L0 (gate,p) region [:, p, :]."""
                    nc.tensor.matmul(
                        reg, WT_ih0[:, mo : mo + 128], xt,
                        start=True, stop=False, skip_group_check=True,
                    )
                    nc.tensor.matmul(
                        reg, bp0[:, :, mo : mo + 128], om,
                        start=False, stop=False, perf_mode=DR,
                        skip_group_check=True,
                    )
                    if COMP_W:
                        for j in range(2):
                            nc.tensor.matmul(
                                reg,
                                wr0[:, 2 * j : 2 * j + 2, mo : mo + 128],
                                hq[:, 0, 2 * j : 2 * j + 2, :],
                                start=False, stop=False, perf_mode=DR,
                                skip_group_check=True,
                            )
                    for k in range(NKC):
                        nc.tensor.matmul(
                            reg,
                            wq0[:, k, :, mo : mo + 128],
                            hq[:, :, k, :],
                            start=False, stop=(k == NKC - 1), perf_mode=DR,
                            skip_group_check=True,
                        )

                def chain_l1(reg, mo, h0q, h1q):
                    nc.tensor.matmul(
                        reg, bp1[:, :, mo : mo + 128], om,
                        start=True, stop=False, perf_mode=DR,
                        skip_group_check=True,
                    )
                    if COMP_W:
                        for j in range(2):
                            nc.tensor.matmul(
                                reg,
                                wr_ih1[:, 2 * j : 2 * j + 2, mo : mo + 128],
                                h0q[:, 0, 2 * j : 2 * j + 2, :],
                                start=False, stop=False, perf_mode=DR,
                                skip_group_check=True,
                            )
                            nc.tensor.matmul(
                                reg,
                                wr_hh1[:, 2 * j : 2 * j + 2, mo : mo + 128],
                                h1q[:, 0, 2 * j : 2 * j + 2, :],
                                start=False, stop=False, perf_mode=DR,
                                skip_group_check=True,
                            )
                    for k in range(NKC):
                        nc.tensor.matmul(
                            reg,
                            wq_ih1[:, k, :, mo : mo + 128],
                            h0q[:, :, k, :],
                            start=False, stop=False, perf_mode=DR,
                            skip_group_check=True,
                        )
                    for k in range(NKC):
                        nc.tensor.matmul(
                            reg,
                            wq_hh1[:, k, :, mo : mo + 128],
                            h1q[:, :, k, :],
                            start=False, stop=(k == NKC - 1), perf_mode=DR,
                            skip_group_check=True,
                        )

                def emit_step(lname, t):
                    """One LSTM layer timestep; yields between gate tiles."""
                    if lname == "0":
                        if xts_sb[t] is not None:
                            xt = xts_sb[t]
                        else:
                            xt = xtp.tile([128, NB], F32R, tag="xt", name="xt")
                            nc.sync.dma_start(out=xt, in_=xts_d[t][:, :])
                        hq = state["h0"]
                    else:
                        h0q = hs0.pop(t)
                        h1q = state["h1"]
                    c_prev = state[f"c{lname}"]
                    sg = {}
                    # gate emission order: g (tanh gate) first, then i, f, o
                    for gname in ("g", "i", "f", "o"):
                        mo_base = GIDX[gname] * NKC * 128
                        ps = psg.tile([128, NKC, NB], F32, tag="g", name=f"ps{lname}")
                        for p in range(NKC):
                            reg = ps[:, p, :]
                            mo = mo_base + p * 128
                            if lname == "0":
                                chain_l0(reg, mo, xt, hq)
                            else:
                                chain_l1(reg, mo, h0q, h1q)
                        psf = ps.rearrange("p a b -> p (a b)")
                        o = wkp.tile(
                            [128, NKC * NB], BF16, tag=f"sg{lname}{gname}",
                            name=f"sg{lname}{gname}",
                        )
                        nc.scalar.activation(
                            o, psf,
                            AF.Tanh if gname == "g" else AF.Sigmoid,
                            scale=1.0 / S,
                        )
                        sg[gname] = o
                        if gname == "i":
                            u = wkp.tile(
                                [128, NKC * NB], BF16, tag=f"u{lname}",
                                name=f"u{lname}",
                            )
                            nc.vector.tensor_mul(u, sg["i"], sg["g"])
                        elif gname == "f":
                            v = wkp.tile(
                                [128, NKC * NB], F32, tag=f"v{lname}",
                                name=f"v{lname}", bufs=1,
                            )
                            nc.gpsimd.tensor_mul(v, sg["f"], c_prev)
                            cn = stp.tile(
                                [128, NKC * NB], F32, tag=f"c{lname}", bufs=2
                            )
                            nc.gpsimd.tensor_add(cn, u, v)
                            th = wkp.tile(
                                [128, NKC * NB], BF16, tag=f"th{lname}",
                                name=f"th{lname}",
                            )
                            nc.scalar.activation(th, cn, AF.Tanh)
                            state[f"c{lname}"] = cn
                        yield
                    hb = wkp.tile(
                        [128, NKC * NB], BF16, tag=f"hb{lname}", name=f"hb{lname}"
                    )
                    nc.vector.tensor_mul(hb, sg["o"], th)
                    hn = stp.tile(
                        [128, 2, NKC, NB], F8, tag=f"h{lname}",
                        name=f"h{lname}", bufs=(LAG + 2) if lname == "0" else 2,
                    )
                    hbv = hb.rearrange("p (a b) -> p a b", a=NKC)
                    nc.vector.tensor_copy(hn[:, 0, :, :], hbv)
                    if COMP_A:
                        d = wkp.tile(
                            [128, NKC * NB], BF16, tag=f"u{lname}", name=f"d{lname}"
                        )
                        nc.vector.tensor_sub(
                            d, hb, hn[:, 0, :, :].rearrange("p a b -> p (a b)")
                        )
                        nc.vector.tensor_scalar_mul(
                            hn[:, 1, :, :],
                            d.rearrange("p (a b) -> p a b", a=NKC),
                            16.0,
                        )
                    else:
                        nc.vector.memset(hn[:, 1, :, :], 0.0)
                    state[f"h{lname}"] = hn
                    if lname == "0":
                        hs0[t] = hn
                    else:
                        state["hb1"] = hb
                    yield

                def drive(gens):
                    alive = list(gens)
                    while alive:
                        for g in list(alive):
                            try:
                                next(g)
                            except StopIteration:
                                alive.remove(g)

                # layer-0 head start; stream in L1 weights meanwhile
                for t in range(LAG):
                    drive([emit_step("0", t)])
                wq_ih1, wr_ih1 = load_w_fp8(W_ih1, "ih1")
                wq_hh1, wr_hh1 = load_w_fp8(W_hh1, "hh1")
                for t in range(LAG, T):
                    drive([emit_step("0", t), emit_step("1", t - LAG)])
                for t in range(T - LAG, T):
                    drive([emit_step("1", t)])

                # ---- head: out.T [10, 256] = W_out @ h1T + b_out ----
                WT_out = const.tile([128, NKC * OUT], BF16, tag="wout")
                stw = wstg.tile([OUT, H], F32, tag="st", name="st_wo")
                nc.scalar.dma_start(out=stw, in_=W_out[:, :])
                for kc in range(NKC):
                    pt = psg.tile([128, NKC, NB], F32, tag="g", name="wptO")
                    ptf = pt.rearrange("p a b -> p (a b)")[:, 0:OUT]
                    nc.tensor.transpose(
                        ptf, stw[:, kc * 128 : (kc + 1) * 128], ident[:OUT, :OUT]
                    )
                    nc.vector.tensor_copy(WT_out[:, kc * OUT : (kc + 1) * OUT], ptf)
                hb1 = state["hb1"]
                psf = psg.tile([128, NKC, NB], F32, tag="g", name="psfin")
                pso = psf.rearrange("p a b -> p (a b)")[:OUT, 0:NB]
                for kc in range(NKC):
                    nc.tensor.matmul(
                        pso,
                        WT_out[:, kc * OUT : (kc + 1) * OUT],
                        hb1[:, kc * NB : (kc + 1) * NB],
                        start=(kc == 0),
                        stop=(kc == NKC - 1),
                        skip_group_check=True,
                    )
                fo = wkp.tile([128, NB], F32, tag="fo")
                nc.vector.tensor_scalar_add(fo[:OUT, :], pso, bot[:, 0:1])
                nc.gpsimd.dma_start(
                    out=out[:, :].rearrange("b o -> o b"), in_=fo[:OUT, :]
                )

    nc.compile()
    return nc


def kernel(**inputs):
    if "nc" not in _CACHE:
        _CACHE["nc"] = _build()
    nc = _CACHE["nc"]

    xb = np.asarray(inputs["xb"], dtype=np.float32)
    shared = {
        k: np.ascontiguousarray(np.asarray(inputs[k], dtype=np.float32))
        for k in (
            "W_ih0",
            "W_hh0",
            "b0",
            "W_ih1",
            "W_hh1",
            "b1",
            "W_out",
            "b_out",
        )
    }
    in_maps = []
    for i in range(NCORES):
        m = dict(shared)
        m["xb"] = np.ascontiguousarray(xb[i * BL : (i + 1) * BL])
        in_maps.append(m)

    trace = False
    try:
        trace = bool(int(os.environ.get("KERNEL_TRACE", "0")))
    except ValueError:
        pass
    try:
        res = run_bass_kernel_spmd(nc, in_maps, list(range(NCORES)), trace=trace)
    except ModuleNotFoundError:
        res = run_bass_kernel_spmd(nc, in_maps, list(range(NCORES)))
    if trace:
        _CACHE["exec_time_ns"] = res.exec_time_ns
    return np.concatenate(
        [res.results[i]["out"] for i in range(NCORES)], axis=0
    )
